# revision 14
# baseline (speedup 1.0000x reference)
"""Distributed multi-head attention (RoPE, non-causal) for 8 TRN2 NeuronCores.

Problem: B=2, S=2048, DIM=768, H=12, HEAD_DIM=64, f32 I/O.

Sharding: 24 (batch, head) pairs -> core c handles batch c//4 and the 3
heads 3*(c%4) .. 3*(c%4)+2.  Per core: QKV projection (bf16 matmuls, f32
PSUM), RoPE (deinterleaved channel layout so rotate_half is contiguous
partition-block copies), scoresT = kT.T @ qT per head (keys on psum
partitions), exp on the scalar engine straight out of PSUM (scale=1/8
folded in, no max-subtraction needed: |scores*scale| < ~0.6), out^T
accumulated via lhsT=[v | ones] so softmax denominators fall out as psum
row 64, normalization via a K=1 broadcast matmul + one multiply, then an
AllToAll over each batch's 4-core group hands every core the full 768
attention channels for its own 512-query slice, and a local output
projection (+b_proj via a K=1 matmul) produces a [512, 768] f32 slice.
Host side only shards/casts inputs and concatenates the 8 output slices.
"""

import os
import sys

sys.path.insert(0, "/opt/trn_rl_repo")

import numpy as np
import ml_dtypes

import concourse.bass as bass
import concourse.mybir as mybir
import concourse.tile as tile
from concourse import bacc, bass_utils

BF16 = mybir.dt.bfloat16
F32 = mybir.dt.float32
AF = mybir.ActivationFunctionType

B, S, DIM, H, DH = 2, 2048, 768, 12, 64
THETA = 10000.0
N_CORES = 8
GROUPS = [[0, 1, 2, 3], [4, 5, 6, 7]]
HL = 3          # heads per core
SC = S // 4     # per-core output row slice (512)
KC = DIM // 128  # 6 contraction chunks

_CACHED = {}


def _build():
    """Build the SPMD Bacc graph (identical on all 8 cores)."""
    nc = bacc.Bacc(None, target_bir_lowering=False)

    xT = nc.declare_dram_parameter("xT", [DIM, S], BF16, isOutput=False)
    wqk = nc.declare_dram_parameter("wqk", [DIM, 2 * HL * DH], BF16, isOutput=False)
    wv = nc.declare_dram_parameter("wv", [DIM, HL * DH], BF16, isOutput=False)
    cosq = nc.declare_dram_parameter("cosq", [128, S], F32, isOutput=False)
    sinq = nc.declare_dram_parameter("sinq", [128, S], F32, isOutput=False)
    wp = nc.declare_dram_parameter("wp", [DIM, DIM], BF16, isOutput=False)
    bp = nc.declare_dram_parameter("bp", [1, DIM], F32, isOutput=False)
    msk = nc.declare_dram_parameter("msk", [DH, N_CORES], F32, isOutput=False)
    out_d = nc.declare_dram_parameter("out", [SC, DIM], F32, isOutput=True)

    with tile.TileContext(nc) as tc:
        with (
            tc.tile_pool(name="const", bufs=1) as const,
            tc.tile_pool(name="work", bufs=2) as work,
            tc.tile_pool(name="psum", bufs=2, space="PSUM") as psum,
            tc.tile_pool(name="dram", bufs=1, space="DRAM") as dram,
        ):
            # ---- load inputs ------------------------------------------------
            xT_sb = const.tile([128, KC, S], BF16)
            wqk_sb = const.tile([128, KC, 2 * HL * DH], BF16)
            wv_sb = const.tile([128, KC, HL * DH], BF16)
            wp_sb = const.tile([128, KC, DIM], BF16)
            cos_sb = const.tile([128, S], F32)
            sin_sb = const.tile([128, S], F32)
            bp_sb = const.tile([1, DIM], F32)
            for k in range(KC):
                nc.gpsimd.dma_start(xT_sb[:, k, :], xT[k * 128:(k + 1) * 128, :])
                nc.gpsimd.dma_start(wqk_sb[:, k, :], wqk[k * 128:(k + 1) * 128, :])
                nc.gpsimd.dma_start(wv_sb[:, k, :], wv[k * 128:(k + 1) * 128, :])
                nc.gpsimd.dma_start(wp_sb[:, k, :], wp[k * 128:(k + 1) * 128, :])
            nc.gpsimd.dma_start(cos_sb[:], cosq[:])
            nc.gpsimd.dma_start(sin_sb[:], sinq[:])
            nc.gpsimd.dma_start(bp_sb[:], bp[:])
            msk_sb = const.tile([DH, N_CORES], F32)
            nc.gpsimd.dma_start(msk_sb[:], msk[:])

            ones_f = const.tile([1, 128], F32)
            nc.vector.memset(ones_f[:], 1.0)

            # ---- qk^T = wqk.T @ xT : [384, S] as [128, 3, S], fused RoPE ---
            # rotate_half swaps 32-partition halves within each 64-row head;
            # the DVE is partition-locked, so the swap copies go over DMA.
            QKM = 2 * HL * DH // 128  # 3 M-blocks
            qkb = const.tile([128, QKM, S], BF16)
            for mb in range(QKM):
                for sb in range(S // 512):
                    sl = slice(sb * 512, (sb + 1) * 512)
                    ps = psum.tile([128, 512], F32, tag="ps_mm")
                    for k in range(KC):
                        nc.tensor.matmul(
                            ps[:],
                            wqk_sb[:, k, mb * 128:(mb + 1) * 128],
                            xT_sb[:, k, sl],
                            start=(k == 0), stop=(k == KC - 1),
                        )
                    qks = work.tile([128, 512], F32, tag="qks", bufs=3)
                    nc.vector.tensor_copy(qks[:], ps[:])
                    rot = work.tile([128, 512], F32, tag="rot")
                    for g in range(2):
                        o = g * 64
                        nc.gpsimd.dma_start(rot[o:o + 32, :], qks[o + 32:o + 64, :])
                        nc.gpsimd.dma_start(rot[o + 32:o + 64, :], qks[o:o + 32, :])
                    tmp = work.tile([128, 512], F32, tag="tmp")
                    nc.vector.tensor_mul(tmp[:], qks[:], cos_sb[:, sl])
                    rots = work.tile([128, 512], F32, tag="rots")
                    nc.vector.tensor_mul(rots[:], rot[:], sin_sb[:, sl])
                    nc.vector.tensor_add(qkb[:, mb, sl], tmp[:], rots[:])

            # ---- v in [s, d] orientation, packed as [v | 1] per head -------
            # v_aug[:, st, 65*h : 65*h+64] = v rows, col 65*h+64 = ones
            v_aug = const.tile([128, S // 128, HL * 65], BF16)
            nc.vector.memset(v_aug[:], 1.0)
            for st in range(S // 128):
                ps = psum.tile([128, HL * DH], F32, tag="ps_mm")
                for k in range(KC):
                    nc.tensor.matmul(
                        ps[:],
                        xT_sb[:, k, st * 128:(st + 1) * 128],
                        wv_sb[:, k, :],
                        start=(k == 0), stop=(k == KC - 1),
                    )
                dst = v_aug[:, st, :].rearrange("p (h x) -> p h x", h=HL)[:, :, 0:DH]
                src = ps.rearrange("p (h x) -> p h x", h=HL)
                nc.vector.tensor_copy(dst, src)

            # head 2's k lives at partition offset 64 but its q at offset 0;
            # matmul needs equal base partitions, so keep a copy of k2 at 0
            # (DMA: partition-shifting copy).
            k2x = const.tile([64, S], BF16)
            nc.gpsimd.dma_start(k2x[:], qkb[64:128, 2, :])

            # ---- attention per local head ----------------------------------
            # 8-core AllToAll, group-masked: my out^T for query block ib goes
            # into shards ib and ib+4, each multiplied by msk[:, d] (1.0 iff
            # destination d is in my batch group, else 0).  After the A2A,
            # shard r holds rank r's heads for MY query block when r is in my
            # group and zeros otherwise, so combined rows = shard[c] +
            # shard[c+768] are exactly W_proj's channel order.  This keeps the
            # SPMD program rank-independent.
            a2a_in = dram.tile([N_CORES * HL * DH, SC], BF16)
            a2a_out = dram.tile([N_CORES * HL * DH, SC], BF16)

            # column order in wqk: [q0, q1 | k0, k1 | q2, k2]
            q_loc = [(0, 0), (0, 64), (2, 0)]   # (m-block, partition offset)
            k_loc = [(1, 0), (1, 64), (2, 64)]
            for h in range(HL):
                qmb, qo = q_loc[h]
                kmb, ko = k_loc[h]
                for ib in range(4):
                    P = work.tile([128, S // 128, 512], BF16, tag="P")
                    for jc in range(S // 128):
                        ps_s = psum.tile([128, 512], F32, tag="ps_s", bufs=3)
                        k_ap = (
                            k2x[:, jc * 128:(jc + 1) * 128]
                            if h == 2
                            else qkb[ko:ko + DH, kmb, jc * 128:(jc + 1) * 128]
                        )
                        nc.tensor.matmul(
                            ps_s[:],
                            k_ap,
                            qkb[qo:qo + DH, qmb, ib * 512:(ib + 1) * 512],
                            start=True, stop=True,
                        )
                        nc.scalar.activation(
                            P[:, jc, :], ps_s[:], AF.Exp, scale=DH ** -0.5
                        )
                    ps_o = psum.tile([DH + 1, 512], F32, tag="ps_o")
                    for jc in range(S // 128):
                        nc.tensor.matmul(
                            ps_o[:],
                            v_aug[:, jc, 65 * h:65 * h + 65],
                            P[:, jc, :],
                            start=(jc == 0), stop=(jc == S // 128 - 1),
                        )
                    rcp = work.tile([1, 512], F32, tag="rcp")
                    nc.vector.reciprocal(rcp[:], ps_o[DH:DH + 1, :])
                    ps_b = psum.tile([DH, 512], F32, tag="ps_b", bufs=1)
                    nc.tensor.matmul(
                        ps_b[:], ones_f[0:1, 0:DH], rcp[:], start=True, stop=True
                    )
                    onum = work.tile([DH, 512], F32, tag="onum")
                    nc.vector.tensor_copy(onum[:], ps_o[0:DH, :])
                    ob = work.tile([DH, 512], BF16, tag="ob")
                    nc.vector.tensor_mul(ob[:], onum[:], ps_b[:])
                    for d in (ib, ib + 4):
                        obm = work.tile([DH, 512], BF16, tag="obm")
                        nc.vector.tensor_scalar_mul(
                            obm[:], ob[:], msk_sb[:, d:d + 1]
                        )
                        r0 = d * HL * DH + h * DH
                        nc.gpsimd.dma_start(a2a_in[r0:r0 + DH, :], obm[:])

            nc.gpsimd.collective_compute(
                "AllToAll",
                mybir.AluOpType.bypass,
                replica_groups=[list(range(N_CORES))],
                ins=[a2a_in.opt()],
                outs=[a2a_out.opt()],
            )

            # ---- output projection on my 512-row slice ---------------------
            ag_sb = const.tile([128, KC, SC], BF16)
            for k in range(KC):
                t1 = work.tile([128, SC], BF16, tag="agh", bufs=4)
                t2 = work.tile([128, SC], BF16, tag="agh", bufs=4)
                nc.gpsimd.dma_start(t1[:], a2a_out[k * 128:(k + 1) * 128, :])
                nc.gpsimd.dma_start(
                    t2[:], a2a_out[(k + KC) * 128:(k + KC + 1) * 128, :]
                )
                nc.vector.tensor_add(ag_sb[:, k, :], t1[:], t2[:])
            ones_b = const.tile([1, 128], BF16)
            nc.vector.memset(ones_b[:], 1.0)

            for m in range(SC // 128):
                for ob_i, (o0, on) in enumerate([(0, 512), (512, 256)]):
                    ps_p = psum.tile([128, on], F32, tag="ps_mm")
                    for k in range(KC):
                        nc.tensor.matmul(
                            ps_p[:],
                            ag_sb[:, k, m * 128:(m + 1) * 128],
                            wp_sb[:, k, o0:o0 + on],
                            start=(k == 0), stop=False,
                        )
                    nc.tensor.matmul(
                        ps_p[:], ones_f[0:1, 0:128], bp_sb[0:1, o0:o0 + on],
                        start=False, stop=True,
                    )
                    po = work.tile([128, on], F32, tag="po")
                    nc.vector.tensor_copy(po[:], ps_p[:])
                    nc.gpsimd.dma_start(
                        out_d[m * 128:(m + 1) * 128, o0:o0 + on], po[:]
                    )

    nc.compile()
    return nc


def _rope_tables():
    inv = (1.0 / (THETA ** (np.arange(0, DH, 2, dtype=np.float32) / DH))).astype(
        np.float32
    )
    pos = np.arange(S, dtype=np.float32)
    f = pos[:, None] * inv[None, :]           # [S, 32] f32, matches reference
    c = np.cos(f).T.astype(np.float32)        # [32, S]
    s = np.sin(f).T.astype(np.float32)
    cos64 = np.concatenate([c, c], axis=0)    # rows i and 32+i = cos(f_i)
    sin64 = np.concatenate([-s, s], axis=0)   # sign folded for rotate_half
    return (
        np.concatenate([cos64, cos64], axis=0),   # [128, S] (two heads/block)
        np.concatenate([sin64, sin64], axis=0),
    )


def _shard_inputs(x, W_qkv, W_proj, b_proj):
    bf16 = ml_dtypes.bfloat16
    cos128, sin128 = _rope_tables()
    # deinterleave perm: new[i] = orig[2i] (i<32), new[32+i] = orig[2i+1]
    perm = np.concatenate([np.arange(0, DH, 2), np.arange(1, DH, 2)])
    wp_t = np.ascontiguousarray(W_proj.T).astype(bf16)          # [c, o]
    bp_r = np.ascontiguousarray(b_proj[None, :]).astype(np.float32)
    in_maps = []
    for c in range(N_CORES):
        b, g = c // 4, c % 4
        heads = range(HL * g, HL * g + HL)
        mask = np.zeros((DH, N_CORES), dtype=np.float32)
        mask[:, 4 * b:4 * b + 4] = 1.0
        hs = list(heads)
        q_r = [h * DH + perm for h in hs]
        k_r = [DIM + h * DH + perm for h in hs]
        # column order [q0, q1 | k0, k1 | q2, k2] to align base partitions
        qk_rows = np.concatenate([q_r[0], q_r[1], k_r[0], k_r[1], q_r[2], k_r[2]])
        v_rows = np.concatenate([2 * DIM + h * DH + np.arange(DH) for h in hs])
        in_maps.append({
            "xT": np.ascontiguousarray(x[b].T).astype(bf16),
            "wqk": np.ascontiguousarray(W_qkv[qk_rows].T).astype(bf16),
            "wv": np.ascontiguousarray(W_qkv[v_rows].T).astype(bf16),
            "cosq": cos128,
            "sinq": sin128,
            "wp": wp_t,
            "bp": bp_r,
            "msk": mask,
        })
    return in_maps


def run(inputs, trace=False, tmpdir=None):
    if "nc" not in _CACHED:
        _CACHED["nc"] = _build()
    nc = _CACHED["nc"]
    in_maps = _shard_inputs(
        inputs["x"], inputs["W_qkv"], inputs["W_proj"], inputs["b_proj"]
    )
    res = bass_utils.run_bass_kernel_spmd(
        nc, in_maps, core_ids=list(range(N_CORES)), trace=trace, tmpdir=tmpdir
    )
    out = np.empty((B, S, DIM), dtype=np.float32)
    for c in range(N_CORES):
        b, g = c // 4, c % 4
        out[b, g * SC:(g + 1) * SC, :] = res.results[c]["out"]
    return out, res


def kernel(**inputs):
    out, _ = run(inputs, trace=False)
    return out


# revision 17
# speedup vs baseline: 1.5969x; 1.5969x over previous
"""Distributed multi-head attention (RoPE, non-causal) for 8 TRN2 NeuronCores.

Problem: B=2, S=2048, DIM=768, H=12, HEAD_DIM=64, f32 I/O.

Sharding: 24 (batch, head) pairs -> core c handles batch c//4 and heads
3*(c%4) .. 3*(c%4)+2.  Per core (bf16 matmuls, f32 PSUM):
  * QKV projection with RoPE fused on the way out of PSUM (deinterleaved
    channel layout so rotate_half is a partition-block swap, done via DMA
    since the DVE is partition-locked).
  * scoresT = kT.T @ qT per head with keys on psum partitions; K=64
    matmuls run as 64x64 quadrant pairs (tile_position) so two j-chunks
    stream concurrently; q/k are stored duplicated on both partition
    halves to feed the row quadrants.
  * exp on the scalar engine straight out of 2-bank PSUM tiles
    (scale=1/8 folded in; scores*scale is bounded ~0.6 so no
    max-subtraction is needed); out^T accumulated via lhsT=[v | ones] so
    softmax denominators fall out as psum row 64; normalization defers
    to a K=1 broadcast matmul + one multiply (reciprocal_approx_fast).
  * One 4-core-group AllGather per 512-query block, issued as soon as
    that block's heads finish (overlaps later blocks' compute), into a
    stacked [4*768, 512] buffer; each core then projects only its own
    512-row slice, located with a host-supplied row offset read into a
    register (dynamic DMA slice) -- the SPMD program stays identical on
    all cores.  b_proj enters via a K=1 ones matmul.
Host side only shards/permutes/casts inputs and concatenates the 8
output slices.
"""

import os
import sys

sys.path.insert(0, "/opt/trn_rl_repo")

import numpy as np
import ml_dtypes

import concourse.bass as bass
import concourse.mybir as mybir
import concourse.tile as tile
from concourse import bacc, bass_utils
from concourse.bass import ds

BF16 = mybir.dt.bfloat16
F32 = mybir.dt.float32
AF = mybir.ActivationFunctionType

B, S, DIM, H, DH = 2, 2048, 768, 12, 64
THETA = 10000.0
N_CORES = 8
GROUPS = [[0, 1, 2, 3], [4, 5, 6, 7]]
HL = 3           # heads per core
SC = S // 4      # per-core output row slice (512)
KC = DIM // 128  # 6 contraction chunks
NJ = S // 128    # 16 key chunks
PACK = os.environ.get("KERNEL_NOPACK") != "1"

_CACHED = {}


def _build():
    """Build the SPMD Bacc graph (identical on all 8 cores)."""
    nc = bacc.Bacc(None, target_bir_lowering=False)

    xT = nc.declare_dram_parameter("xT", [DIM, S], BF16, isOutput=False)
    wqk = nc.declare_dram_parameter("wqk", [DIM, 2 * HL * DH], BF16, isOutput=False)
    wv = nc.declare_dram_parameter("wv", [DIM, HL * DH], BF16, isOutput=False)
    cosq = nc.declare_dram_parameter("cosq", [128, S], F32, isOutput=False)
    sinq = nc.declare_dram_parameter("sinq", [128, S], F32, isOutput=False)
    wp = nc.declare_dram_parameter("wp", [DIM, DIM], BF16, isOutput=False)
    bp = nc.declare_dram_parameter("bp", [1, DIM], F32, isOutput=False)
    soff = nc.declare_dram_parameter("soff", [1, 1], mybir.dt.uint32, isOutput=False)
    out_d = nc.declare_dram_parameter("out", [SC, DIM], F32, isOutput=True)

    with tile.TileContext(nc) as tc:
        with (
            tc.tile_pool(name="const", bufs=1) as const,
            tc.tile_pool(name="work", bufs=2) as work,
            tc.tile_pool(name="psum", bufs=2, space="PSUM") as psum,
            tc.tile_pool(name="dram", bufs=1, space="DRAM") as dram,
        ):
            # ---- load inputs ------------------------------------------------
            xT_sb = const.tile([128, KC, S], BF16)
            wqk_sb = const.tile([128, KC, 2 * HL * DH], BF16)
            wv_sb = const.tile([128, KC, HL * DH], BF16)
            wp_sb = const.tile([128, KC, DIM], BF16)
            cos_sb = const.tile([128, S], F32)
            sin_sb = const.tile([128, S], F32)
            bp_sb = const.tile([1, DIM], F32)
            for k in range(KC):
                nc.gpsimd.dma_start(xT_sb[:, k, :], xT[k * 128:(k + 1) * 128, :])
                nc.gpsimd.dma_start(wqk_sb[:, k, :], wqk[k * 128:(k + 1) * 128, :])
                nc.gpsimd.dma_start(wv_sb[:, k, :], wv[k * 128:(k + 1) * 128, :])
                nc.gpsimd.dma_start(wp_sb[:, k, :], wp[k * 128:(k + 1) * 128, :])
            nc.gpsimd.dma_start(cos_sb[:], cosq[:])
            nc.gpsimd.dma_start(sin_sb[:], sinq[:])
            nc.gpsimd.dma_start(bp_sb[:], bp[:])

            ones_f = const.tile([1, 128], F32)
            nc.vector.memset(ones_f[:], 1.0)

            # ---- qk^T = wqk.T @ xT with fused RoPE -------------------------
            # wqk column order [q0, q1 | k0, k1 | q2, k2], channels
            # deinterleaved per head so rotate_half = swap 32-row halves.
            QKM = 2 * HL * DH // 128  # 3 M-blocks
            qkb = const.tile([128, QKM, S], BF16)
            for mb in range(QKM):
                for sb in range(S // 512):
                    sl = slice(sb * 512, (sb + 1) * 512)
                    ps = psum.tile([128, 512], F32, tag="ps_mm")
                    for k in range(KC):
                        nc.tensor.matmul(
                            ps[:],
                            wqk_sb[:, k, mb * 128:(mb + 1) * 128],
                            xT_sb[:, k, sl],
                            start=(k == 0), stop=(k == KC - 1),
                        )
                    qks = work.tile([128, 512], F32, tag="qks", bufs=3)
                    nc.vector.tensor_copy(qks[:], ps[:])
                    rot = work.tile([128, 512], F32, tag="rot")
                    for g in range(2):
                        o = g * 64
                        nc.gpsimd.dma_start(rot[o:o + 32, :], qks[o + 32:o + 64, :])
                        nc.gpsimd.dma_start(rot[o + 32:o + 64, :], qks[o:o + 32, :])
                    tmp = work.tile([128, 512], F32, tag="tmp")
                    nc.vector.tensor_mul(tmp[:], qks[:], cos_sb[:, sl])
                    rots = work.tile([128, 512], F32, tag="rots")
                    nc.vector.tensor_mul(rots[:], rot[:], sin_sb[:, sl])
                    nc.vector.tensor_add(qkb[:, mb, sl], tmp[:], rots[:])

            # per-head q/k, duplicated on both partition halves (feeds the
            # two PE row quadrants when packing; DMA = partition shift)
            q_loc = [(0, 0), (0, 64), (2, 0)]   # (m-block, partition offset)
            k_loc = [(1, 0), (1, 64), (2, 64)]
            qh2, kh2 = [], []
            if PACK:
                for h in range(HL):
                    qt = const.tile([128, S], BF16, tag=f"qh2_{h}")
                    kt = const.tile([128, S], BF16, tag=f"kh2_{h}")
                    (qmb, qo), (kmb, ko) = q_loc[h], k_loc[h]
                    for half in range(2):
                        o = half * 64
                        nc.gpsimd.dma_start(
                            qt[o:o + 64, :], qkb[qo:qo + DH, qmb, :]
                        )
                        nc.gpsimd.dma_start(
                            kt[o:o + 64, :], qkb[ko:ko + DH, kmb, :]
                        )
                    qh2.append(qt)
                    kh2.append(kt)
            else:
                k2x = const.tile([64, S], BF16)
                nc.gpsimd.dma_start(k2x[:], qkb[64:128, 2, :])

            # ---- v in [s, d] orientation, packed as [v | 1] per head -------
            v_aug = const.tile([128, NJ, HL * 65], BF16)
            nc.vector.memset(v_aug[:], 1.0)
            for st in range(NJ):
                ps = psum.tile([128, HL * DH], F32, tag="ps_mm")
                for k in range(KC):
                    nc.tensor.matmul(
                        ps[:],
                        xT_sb[:, k, st * 128:(st + 1) * 128],
                        wv_sb[:, k, :],
                        start=(k == 0), stop=(k == KC - 1),
                    )
                dst = v_aug[:, st, :].rearrange("p (h x) -> p h x", h=HL)[:, :, 0:DH]
                src = ps.rearrange("p (h x) -> p h x", h=HL)
                nc.vector.tensor_copy(dst, src)

            # ---- attention: ib outer so each block's AllGather overlaps ----
            ag_out4 = dram.tile([4 * DIM, SC], BF16)
            scale = DH ** -0.5
            for ib in range(4):
                isl = slice(ib * 512, (ib + 1) * 512)
                ag_in = dram.tile([HL * DH * 4 // 4, SC], BF16, tag=f"agin{ib}")
                for h in range(HL):
                    P = work.tile([128, NJ, 512], BF16, tag="P")
                    for t in range(NJ // 2):
                        ps2 = psum.tile([128, 2, 512], F32, tag="ps_s")
                        j0, j1 = 2 * t, 2 * t + 1
                        if PACK:
                            qt, kt = qh2[h], kh2[h]
                            nc.tensor.matmul(
                                ps2[0:64, 0, :],
                                kt[0:64, j0 * 128:j0 * 128 + 64],
                                qt[0:64, isl], start=True, stop=True,
                                tile_position=(0, 0),
                            )
                            nc.tensor.matmul(
                                ps2[64:128, 0, :],
                                kt[0:64, j0 * 128 + 64:(j0 + 1) * 128],
                                qt[0:64, isl], start=True, stop=True,
                                tile_position=(0, 64),
                            )
                            nc.tensor.matmul(
                                ps2[0:64, 1, :],
                                kt[64:128, j1 * 128:j1 * 128 + 64],
                                qt[64:128, isl], start=True, stop=True,
                                tile_position=(64, 0),
                            )
                            nc.tensor.matmul(
                                ps2[64:128, 1, :],
                                kt[64:128, j1 * 128 + 64:(j1 + 1) * 128],
                                qt[64:128, isl], start=True, stop=True,
                                tile_position=(64, 64),
                            )
                        else:
                            (qmb, qo), (kmb, ko) = q_loc[h], k_loc[h]
                            for tt, j in ((0, j0), (1, j1)):
                                k_ap = (
                                    k2x[:, j * 128:(j + 1) * 128]
                                    if h == 2
                                    else qkb[ko:ko + DH, kmb, j * 128:(j + 1) * 128]
                                )
                                nc.tensor.matmul(
                                    ps2[:, tt, :], k_ap,
                                    qkb[qo:qo + DH, qmb, isl],
                                    start=True, stop=True,
                                )
                        nc.scalar.activation(
                            P[:, j0:j0 + 2, :], ps2[:], AF.Exp, scale=scale
                        )
                    ps_o = psum.tile([DH + 1, 512], F32, tag="ps_o")
                    for jc in range(NJ):
                        nc.tensor.matmul(
                            ps_o[:],
                            v_aug[:, jc, 65 * h:65 * h + 65],
                            P[:, jc, :],
                            start=(jc == 0), stop=(jc == NJ - 1),
                        )
                    den = work.tile([1, 512], F32, tag="den")
                    nc.vector.tensor_copy(den[:], ps_o[DH:DH + 1, :])
                    rcp = work.tile([1, 512], F32, tag="rcp")
                    nc.vector.reciprocal_approx_fast(rcp[:], den[:])
                    ps_b = psum.tile([DH, 512], F32, tag="ps_mm")
                    nc.tensor.matmul(
                        ps_b[:], ones_f[0:1, 0:DH], rcp[:], start=True, stop=True
                    )
                    onum = work.tile([DH, 512], F32, tag="onum")
                    nc.vector.tensor_copy(onum[:], ps_o[0:DH, :])
                    ob = work.tile([DH, 512], BF16, tag="ob")
                    nc.vector.tensor_mul(ob[:], onum[:], ps_b[:])
                    nc.gpsimd.dma_start(ag_in[h * DH:(h + 1) * DH, :], ob[:])

                nc.gpsimd.collective_compute(
                    "AllGather",
                    mybir.AluOpType.bypass,
                    replica_groups=GROUPS,
                    ins=[ag_in.opt()],
                    outs=[ag_out4[ib * DIM:(ib + 1) * DIM, :]],
                )

            # ---- output projection on my 512-row slice ---------------------
            with tc.tile_critical():
                reg = nc.gpsimd.alloc_register("soff_reg")
                nc.gpsimd.reg_load(reg, soff[0:1, 0:1])
                sv = nc.gpsimd.snap(reg, donate=True, min_val=0, max_val=3 * DIM)
            ag_sb = const.tile([128, KC, SC], BF16)
            nc.gpsimd.dma_start(
                ag_sb[:],
                ag_out4[ds(sv, DIM), :].rearrange("(k p) n -> p k n", p=128),
            )

            for m in range(SC // 128):
                for o0, on in ((0, 512), (512, 256)):
                    ps_p = psum.tile([128, on], F32, tag="ps_mm")
                    for k in range(KC):
                        nc.tensor.matmul(
                            ps_p[:],
                            ag_sb[:, k, m * 128:(m + 1) * 128],
                            wp_sb[:, k, o0:o0 + on],
                            start=(k == 0), stop=False,
                        )
                    nc.tensor.matmul(
                        ps_p[:], ones_f[0:1, 0:128], bp_sb[0:1, o0:o0 + on],
                        start=False, stop=True,
                    )
                    po = work.tile([128, on], F32, tag="po")
                    nc.vector.tensor_copy(po[:], ps_p[:])
                    nc.gpsimd.dma_start(
                        out_d[m * 128:(m + 1) * 128, o0:o0 + on], po[:]
                    )

    nc.compile()
    return nc


def _rope_tables():
    inv = (1.0 / (THETA ** (np.arange(0, DH, 2, dtype=np.float32) / DH))).astype(
        np.float32
    )
    pos = np.arange(S, dtype=np.float32)
    f = pos[:, None] * inv[None, :]           # [S, 32] f32, matches reference
    c = np.cos(f).T.astype(np.float32)        # [32, S]
    s = np.sin(f).T.astype(np.float32)
    cos64 = np.concatenate([c, c], axis=0)    # rows i and 32+i = cos(f_i)
    sin64 = np.concatenate([-s, s], axis=0)   # sign folded for rotate_half
    return (
        np.concatenate([cos64, cos64], axis=0),   # [128, S] (two heads/block)
        np.concatenate([sin64, sin64], axis=0),
    )


def _shard_inputs(x, W_qkv, W_proj, b_proj):
    bf16 = ml_dtypes.bfloat16
    cos128, sin128 = _rope_tables()
    # deinterleave perm: new[i] = orig[2i] (i<32), new[32+i] = orig[2i+1]
    perm = np.concatenate([np.arange(0, DH, 2), np.arange(1, DH, 2)])
    wp_t = np.ascontiguousarray(W_proj.T).astype(bf16)          # [c, o]
    bp_r = np.ascontiguousarray(b_proj[None, :]).astype(np.float32)
    in_maps = []
    for c in range(N_CORES):
        b, g = c // 4, c % 4
        hs = [HL * g + i for i in range(HL)]
        q_r = [h * DH + perm for h in hs]
        k_r = [DIM + h * DH + perm for h in hs]
        # column order [q0, q1 | k0, k1 | q2, k2] to align base partitions
        qk_rows = np.concatenate([q_r[0], q_r[1], k_r[0], k_r[1], q_r[2], k_r[2]])
        v_rows = np.concatenate([2 * DIM + h * DH + np.arange(DH) for h in hs])
        in_maps.append({
            "xT": np.ascontiguousarray(x[b].T).astype(bf16),
            "wqk": np.ascontiguousarray(W_qkv[qk_rows].T).astype(bf16),
            "wv": np.ascontiguousarray(W_qkv[v_rows].T).astype(bf16),
            "cosq": cos128,
            "sinq": sin128,
            "wp": wp_t,
            "bp": bp_r,
            "soff": np.array([[g * DIM]], dtype=np.uint32),
        })
    return in_maps


def run(inputs, trace=False, tmpdir=None):
    if "nc" not in _CACHED:
        _CACHED["nc"] = _build()
    nc = _CACHED["nc"]
    in_maps = _shard_inputs(
        inputs["x"], inputs["W_qkv"], inputs["W_proj"], inputs["b_proj"]
    )
    res = bass_utils.run_bass_kernel_spmd(
        nc, in_maps, core_ids=list(range(N_CORES)), trace=trace, tmpdir=tmpdir
    )
    out = np.empty((B, S, DIM), dtype=np.float32)
    for c in range(N_CORES):
        b, g = c // 4, c % 4
        out[b, g * SC:(g + 1) * SC, :] = res.results[c]["out"]
    return out, res


def kernel(**inputs):
    out, _ = run(inputs, trace=False)
    return out
